# revision 1
# baseline (speedup 1.0000x reference)
"""DiT block with AdaLN-Zero on 8 Trainium2 NeuronCores.

Strategy: data-parallel over the sequence dimension (S=2048 -> 8 x 256) with a
window_size=16 halo on each side; the banded attention (|i-j|<=16) makes every
core's work fully local, so there are no collectives at all.

On-device layout is feature-major ([D partitions, tokens free]) end to end:
matmuls keep the weights stationary (lhsT = W.T tiles) so activations stay
feature-major through the whole block; LayerNorm statistics are computed with
ones-vector matmuls on the tensor engine and broadcast back across partitions
with rank-1 outer-product matmuls.  Matmul tiles use float32r (full-rate fp32
path for moving dim >= 256, ~1.5e-4 matmul precision).
"""
import sys

sys.path.insert(0, '/opt/trn_rl_repo')

import numpy as np

S, B, D, H, MLP_H, W = 2048, 4, 1024, 16, 4096, 16
NCORES = 8
SS = S // NCORES          # 256 sequence positions owned per core
SE = SS + 2 * W           # 288 with halo
NEXT = B * SE             # 1152 extended tokens per core
NINT = B * SS             # 1024 interior tokens per core
DH = D // H               # 64
EPS = 1e-6
P = 128
DT = D // P               # 8 feature tiles of 128
# token-partition chunks of the extended 288 rows per batch: (128, 128, 32)
KCH = (128, 128, 32)

USE_F32R = True

_BUILD_CACHE = {}


def _build_nc():
    import concourse.bass as bass
    import concourse.tile as tile
    from concourse import bacc, mybir
    from concourse.masks import make_identity

    dt = mybir.dt
    F32 = dt.float32
    MM = dt.float32r if USE_F32R else dt.float32
    AF = mybir.ActivationFunctionType
    MULT = mybir.AluOpType.mult
    ADD = mybir.AluOpType.add

    def R(ap):
        return ap

    nc = bacc.Bacc("TRN2", target_bir_lowering=False, debug=False,
                   num_devices=NCORES)

    # ---- I/O ----
    xT_d = nc.dram_tensor("xT", [D, NEXT], MM, kind="ExternalInput").ap()
    mask_d = nc.dram_tensor("mask", [SE, SS], F32, kind="ExternalInput").ap()
    cTa_d = nc.dram_tensor("cTa", [D + 1, B], MM, kind="ExternalInput").ap()
    adaTa_d = nc.dram_tensor("adaTa", [D + 1, 6 * D], MM, kind="ExternalInput").ap()
    wqkT_d = nc.dram_tensor("wqkT", [D, 2 * D], MM, kind="ExternalInput").ap()
    qkb_d = nc.dram_tensor("qkb", [2 * D], F32, kind="ExternalInput").ap()
    wvT_d = nc.dram_tensor("wvT", [D, D], MM, kind="ExternalInput").ap()
    vb128_d = nc.dram_tensor("vb128", [P, D], F32, kind="ExternalInput").ap()
    woT_d = nc.dram_tensor("woT", [D, D], MM, kind="ExternalInput").ap()
    ob_d = nc.dram_tensor("ob", [D], F32, kind="ExternalInput").ap()
    w1T_d = nc.dram_tensor("w1T", [D, MLP_H], MM, kind="ExternalInput").ap()
    b1_d = nc.dram_tensor("b1", [MLP_H], F32, kind="ExternalInput").ap()
    w2T_d = nc.dram_tensor("w2T", [MLP_H, D], MM, kind="ExternalInput").ap()
    b2_d = nc.dram_tensor("b2", [D], F32, kind="ExternalInput").ap()
    lnr_d = nc.dram_tensor("lnrows", [2, NEXT], MM, kind="ExternalInput").ap()
    yT_d = nc.dram_tensor("yT", [D, NINT], F32, kind="ExternalOutput").ap()

    with tile.TileContext(nc) as tc:
        from contextlib import ExitStack
        with ExitStack() as ctx:
            misc = ctx.enter_context(tc.tile_pool(name="misc", bufs=1))
            dram = ctx.enter_context(tc.tile_pool(name="dram", bufs=1, space="DRAM"))

            # DRAM scratch (SBUF relief)
            oD = dram.tile([D, NINT], MM)
            x2D = dram.tile([D, NINT], MM)

            # ---- constants ----
            ones_f = misc.tile([P, P], F32)
            nc.vector.memset(ones_f[:], 1.0)
            ones_t = misc.tile([P, P], MM)
            nc.scalar.copy(ones_t[:], ones_f[:])
            ident = misc.tile([P, P], F32)
            make_identity(nc, ident[:])
            eps_t = misc.tile([P, 1], F32)
            nc.vector.memset(eps_t[:], EPS)

            mask_t = misc.tile([P, 3, SS], F32)
            qkb_t = misc.tile([P, 16], F32)
            ob_t = misc.tile([P, DT], F32)
            b1_t = misc.tile([P, 32], F32)
            b2_t = misc.tile([P, DT], F32)

            def load_small_consts():
                for j in range(3):
                    nc.sync.dma_start(out=mask_t[0:KCH[j], j, :],
                                      in_=mask_d[j * P:j * P + KCH[j], :])
                nc.sync.dma_start(out=qkb_t[:],
                                  in_=qkb_d.rearrange("(m p) -> p m", p=P))
                nc.sync.dma_start(out=ob_t[:],
                                  in_=ob_d.rearrange("(m p) -> p m", p=P))
                nc.sync.dma_start(out=b1_t[:],
                                  in_=b1_d.rearrange("(m p) -> p m", p=P))
                nc.sync.dma_start(out=b2_t[:],
                                  in_=b2_d.rearrange("(m p) -> p m", p=P))

            crelu_t = misc.tile([P, DT + 1, B], MM)
            zf = misc.tile([P, B], F32)
            nc.vector.memset(zf[:], 0.0)
            nc.scalar.copy(crelu_t[:, DT, :], zf[:])
            nc.sync.dma_start(out=crelu_t[:, 0:DT, :],
                              in_=cTa_d[0:D, :].rearrange("(k p) b -> p k b", p=P))
            nc.sync.dma_start(out=crelu_t[0:1, DT, :], in_=cTa_d[D:D + 1, :])
            nc.scalar.activation(crelu_t[:], crelu_t[:], AF.Relu)

            # modT[:, g, dti, b] = mod group g, feature tile dti, batch b
            modT = misc.tile([P, 6, DT, B], F32)

            # adaLN modulation for feature-column group `nt` (512 wide).
            # groups 0..1 (shift/scale msa) are computed first so LN1 can
            # start; the rest is emitted later and overlaps attention.
            def ada_part(nts, ps_a, mod_pool, ast):
                mod_sb = mod_pool.tile([B, 2 * D], F32, tag="mod_sb")
                for ni, nt in enumerate(nts):
                    pm = ps_a.tile([B, 512], F32, tag="mod")
                    for k in range(DT + 1):
                        at = ast.tile([P, 512], MM, tag="ada")
                        kp = P if k < DT else 1
                        nc.sync.dma_start(
                            out=at[0:kp, :],
                            in_=adaTa_d[k * P:k * P + kp, nt * 512:(nt + 1) * 512])
                        nc.tensor.matmul(out=pm[:], lhsT=R(crelu_t[0:kp, k, :]),
                                         rhs=R(at[0:kp, :]),
                                         start=(k == 0), stop=(k == DT))
                    nc.scalar.copy(mod_sb[:, ni * 512:(ni + 1) * 512], pm[:])
                    if ni % 2 == 1:     # one mod group complete (2 chunks)
                        g = nt // 2
                        for dti in range(DT):
                            pt = ps_a.tile([P, B], F32, tag="tr")
                            col = (ni - 1) * 512 + dti * P
                            nc.tensor.transpose(
                                pt[:], mod_sb[:, col:col + P], ident[0:B, 0:B])
                            nc.scalar.copy(modT[:, g, dti, :], pt[:])

            def modcol(g, dti, b):
                return modT[:, g, dti, b:b + 1]

            # finalize LN stats: srow/qrow hold per-token sum and sum-of-sq
            def ln_finalize(srow, qrow, ntok, rows):
                pbc_cm = tc.tile_pool(name="ps_bc", bufs=2, space="PSUM")
                pbc = pbc_cm.__enter__()
                mb = rows.tile([P, ntok], F32, tag="mb")
                qb = rows.tile([P, ntok], F32, tag="qb")
                t1 = rows.tile([P, ntok], F32, tag="t1")
                for c0 in range(0, ntok, 512):
                    csz = min(512, ntok - c0)
                    pb = pbc.tile([P, 512], F32, tag="bc")
                    nc.tensor.matmul(out=pb[:, 0:csz], lhsT=R(ones_t[0:1, :]),
                                     rhs=R(srow[:, c0:c0 + csz]),
                                     start=True, stop=True)
                    nc.scalar.mul(mb[:, c0:c0 + csz], pb[:, 0:csz], 1.0 / D)
                    pb2 = pbc.tile([P, 512], F32, tag="bc")
                    nc.tensor.matmul(out=pb2[:, 0:csz], lhsT=R(ones_t[0:1, :]),
                                     rhs=R(qrow[:, c0:c0 + csz]),
                                     start=True, stop=True)
                    nc.scalar.mul(qb[:, c0:c0 + csz], pb2[:, 0:csz], 1.0 / D)
                nc.vector.tensor_mul(t1[:], mb[:], mb[:])
                nc.vector.tensor_sub(qb[:], qb[:], t1[:])
                nc.scalar.activation(qb[:], qb[:], AF.Sqrt, bias=eps_t[:])
                nc.vector.reciprocal(t1[:], qb[:])         # rstd
                nc.vector.tensor_mul(mb[:], mb[:], t1[:])  # mean*rstd
                pbc_cm.__exit__(None, None, None)
                return t1, mb

            # ================= main dataflow =================
            with tc.tile_pool(name="v_pool", bufs=1) as v_pool, \
                 tc.tile_pool(name="qk_pool", bufs=1) as qk_pool:
                # per head 65 columns: 64 of v plus a ones column that makes
                # the AV matmul also produce the softmax denominator row
                vtok = v_pool.tile([P, B, 3, H, DH + 1], MM)
                qT = qk_pool.tile([P, DT, NINT], MM)
                kT = qk_pool.tile([P, DT, NEXT], MM)

                with tc.tile_pool(name="xm_pool", bufs=1) as xm_pool:
                    xmT = xm_pool.tile([P, DT, NEXT], MM)

                    # ---- LN1: host-computed per-token rstd / mean*rstd,
                    # broadcast across partitions with rank-1 matmuls ----
                    with tc.tile_pool(name="ln1_rows", bufs=1) as rows, \
                         tc.tile_pool(name="ps_bc1", bufs=2, space="PSUM") as pbc1:
                        lnr_a = rows.tile([1, NEXT], MM, tag="lnra")
                        nc.sync.dma_start(out=lnr_a[:], in_=lnr_d[0:1, :])
                        lnr_m = rows.tile([1, NEXT], MM, tag="lnrm")
                        nc.sync.dma_start(out=lnr_m[:], in_=lnr_d[1:2, :])
                        for dti in range(DT):
                            nc.sync.dma_start(out=xmT[:, dti, :],
                                              in_=xT_d[dti * P:(dti + 1) * P, :])
                        t1 = rows.tile([P, NEXT], F32, tag="t1")
                        mb = rows.tile([P, NEXT], F32, tag="mb")
                        for c0 in range(0, NEXT, 384):
                            pb = pbc1.tile([P, 384], F32, tag="bc")
                            nc.tensor.matmul(out=pb[:], lhsT=R(ones_t[0:1, :]),
                                             rhs=R(lnr_a[:, c0:c0 + 384]),
                                             start=True, stop=True)
                            nc.scalar.copy(t1[:, c0:c0 + 384], pb[:])
                            pb2 = pbc1.tile([P, 384], F32, tag="bc")
                            nc.tensor.matmul(out=pb2[:], lhsT=R(ones_t[0:1, :]),
                                             rhs=R(lnr_m[:, c0:c0 + 384]),
                                             start=True, stop=True)
                            nc.scalar.copy(mb[:, c0:c0 + 384], pb2[:])
                        load_small_consts()
                        with tc.tile_pool(name="ada_sb1", bufs=1) as mp1, \
                             tc.tile_pool(name="ada_st1", bufs=9) as as1, \
                             tc.tile_pool(name="ps_a1", bufs=1, space="PSUM") as pa1:
                            ada_part([0, 1, 2, 3], pa1, mp1, as1)
                        for dti in range(DT):
                            eng = nc.vector
                            eng.tensor_mul(xmT[:, dti, :], xmT[:, dti, :], t1[:])
                            eng.tensor_sub(xmT[:, dti, :], xmT[:, dti, :], mb[:])
                            for b in range(B):
                                sl = slice(b * SE, (b + 1) * SE)
                                nc.scalar.activation(
                                    xmT[:, dti, sl], xmT[:, dti, sl], AF.Identity,
                                    bias=modcol(0, dti, b),
                                    scale=modcol(1, dti, b))

                    # ---- v = xm @ WvT + bv (token-major, per batch) ----
                    with tc.tile_pool(name="wv_pool", bufs=1) as wvp, \
                         tc.tile_pool(name="vb_pool", bufs=1) as vbp, \
                         tc.tile_pool(name="ps_c", bufs=4, space="PSUM") as ps_c:
                        vb_t = vbp.tile([P, D], F32)
                        nc.sync.dma_start(out=vb_t[:], in_=vb128_d)
                        wv_t = wvp.tile([P, DT, D], MM)
                        for k in range(DT):
                            nc.sync.dma_start(
                                out=wv_t[:, k, :], in_=wvT_d[k * P:(k + 1) * P, :])
                        for b in range(B):
                            for j in range(3):
                                csz = KCH[j]
                                t0 = b * SE + j * P
                                nc.scalar.copy(vtok[0:csz, b, j, :, DH:DH + 1],
                                               ones_f[0:csz, 0:H])
                                for nci in range(2):
                                    pv = ps_c.tile([P, 512], F32, tag="v")
                                    for k in range(DT):
                                        nc.tensor.matmul(
                                            out=pv[0:csz, :],
                                            lhsT=R(xmT[:, k, t0:t0 + csz]),
                                            rhs=R(wv_t[:, k, nci * 512:(nci + 1) * 512]),
                                            start=(k == 0), stop=(k == DT - 1))
                                    nc.vector.tensor_add(
                                        vtok[0:csz, b, j, nci * 8:(nci + 1) * 8, 0:DH],
                                        pv[0:csz, :],
                                        vb_t[0:csz, nci * 512:(nci + 1) * 512])

                    # ---- q (interior) and k (ext) -> DRAM ----
                    with tc.tile_pool(name="wqk_pool", bufs=2) as wqkp, \
                         tc.tile_pool(name="ps_d", bufs=4, space="PSUM") as ps_d:
                        for m in range(DT):     # q features
                            wm = wqkp.tile([P, DT, P], MM, tag="w")
                            nc.sync.dma_start(
                                out=wm[:],
                                in_=wqkT_d[:, m * P:(m + 1) * P]
                                .rearrange("(k p) m -> p k m", p=P))
                            for b in range(B):
                                pq = ps_d.tile([P, SS], F32, tag="q")
                                for k in range(DT):
                                    nc.tensor.matmul(
                                        out=pq[:],
                                        lhsT=R(wm[:, k, :]),
                                        rhs=R(xmT[:, k, b * SE + W:b * SE + W + SS]),
                                        start=(k == 0), stop=(k == DT - 1))
                                nc.scalar.activation(
                                    qT[:, m, b * SS:(b + 1) * SS], pq[:],
                                    AF.Identity, bias=qkb_t[:, m:m + 1])
                        for m in range(DT):     # k features
                            wm = wqkp.tile([P, DT, P], MM, tag="w")
                            nc.sync.dma_start(
                                out=wm[:],
                                in_=wqkT_d[:, D + m * P:D + (m + 1) * P]
                                .rearrange("(k p) m -> p k m", p=P))
                            for nci in range(3):
                                pk = ps_d.tile([P, 384], F32, tag="k")
                                for k in range(DT):
                                    nc.tensor.matmul(
                                        out=pk[:],
                                        lhsT=R(wm[:, k, :]),
                                        rhs=R(xmT[:, k, nci * 384:(nci + 1) * 384]),
                                        start=(k == 0), stop=(k == DT - 1))
                                nc.vector.tensor_scalar_add(
                                    kT[:, m, nci * 384:(nci + 1) * 384], pk[:],
                                    qkb_t[:, DT + m:DT + m + 1])

                # ---- banded attention, per (h, b); the remaining four
                # modulation groups and phase-F prefetches are interleaved so
                # their DMA overlaps attention compute ----
                with tc.tile_pool(name="att_sb", bufs=2) as att, \
                     tc.tile_pool(name="ps_sc", bufs=3, space="PSUM") as ps_sc, \
                     tc.tile_pool(name="ps_sm", bufs=1, space="PSUM") as ps_sm:
                    def emit_ada_group(g):
                        mod_sb = att.tile([B, 2, 512], F32, tag="mod_sb", bufs=2)
                        for ni in range(2):
                            nt = 2 * g + ni
                            pm = ps_sm.tile([B, 512], F32, tag="mod", bufs=1)
                            for k in range(DT + 1):
                                at = att.tile([P, 512], MM, tag="ada", bufs=9)
                                kp = P if k < DT else 1
                                nc.sync.dma_start(
                                    out=at[0:kp, :],
                                    in_=adaTa_d[k * P:k * P + kp,
                                                nt * 512:(nt + 1) * 512])
                                nc.tensor.matmul(
                                    out=pm[:], lhsT=R(crelu_t[0:kp, k, :]),
                                    rhs=R(at[0:kp, :]),
                                    start=(k == 0), stop=(k == DT))
                            nc.scalar.copy(mod_sb[:, ni, :], pm[:])
                        for dti in range(DT):
                            pt = ps_sm.tile([P, B], F32, tag="tr", bufs=1)
                            col = dti * P
                            nc.tensor.transpose(
                                pt[:], mod_sb[:, col // 512, col % 512:col % 512 + P],
                                ident[0:B, 0:B])
                            nc.scalar.copy(modT[:, g, dti, :], pt[:])
                    # software-pipelined by one (h, b) pair: the head emits
                    # scores/exp/mask; the tail (AV, normalize, store) of the
                    # previous pair is emitted afterwards so no engine FIFO
                    # head-of-line-blocks on a cross-engine chain.
                    def att_tail(h, b, exps):
                        pso = ps_sm.tile([DH + 1, SS], F32, tag="o", bufs=2)
                        for j in range(3):
                            csz = KCH[j]
                            nc.tensor.matmul(
                                out=pso[:],
                                lhsT=R(vtok[0:csz, b, j, h, :]),
                                rhs=R(exps[j][0:csz, :]),
                                start=(j == 0), stop=(j == 2))
                        # row DH holds sum(exp)
                        o_ev = att.tile([DH + 1, SS], MM, tag="oev", bufs=3)
                        nc.scalar.copy(o_ev[:], pso[:])
                        with nc.allow_low_precision(
                                reason="f32r storage is 32-bit"):
                            nc.vector.reciprocal(o_ev[DH:DH + 1, :],
                                                 o_ev[DH:DH + 1, :])
                        prb = ps_sm.tile([64, SS], F32, tag="rb", bufs=1)
                        nc.tensor.matmul(
                            out=prb[:], lhsT=R(ones_t[DH:DH + 1, 0:64]),
                            rhs=R(o_ev[DH:DH + 1, :]), start=True, stop=True)
                        oatt = att.tile([64, SS], MM, tag="oatt", bufs=4)
                        nc.vector.tensor_mul(oatt[:], o_ev[0:DH, :], prb[:])
                        nc.sync.dma_start(
                            out=oD[h * DH:(h + 1) * DH, b * SS:(b + 1) * SS],
                            in_=oatt[:])

                    pending = None
                    for idx in range(H * B):
                        h, b = idx // B, idx % B
                        if True:
                            ro = DH * (h % 2)
                            mt = h // 2
                            exps = []
                            for j in range(3):
                                csz = KCH[j]
                                pss = ps_sc.tile([P, SS], F32, tag="sc")
                                nc.tensor.matmul(
                                    out=pss[0:csz, :],
                                    lhsT=R(kT[ro:ro + DH, mt,
                                             b * SE + j * P:b * SE + j * P + csz]),
                                    rhs=R(qT[ro:ro + DH, mt, b * SS:(b + 1) * SS]),
                                    start=True, stop=True)
                                et = att.tile([P, SS], MM, tag="exp", bufs=9)
                                nc.scalar.activation(et[0:csz, :], pss[0:csz, :],
                                                     AF.Exp, scale=0.125)
                                nc.vector.tensor_mul(et[0:csz, :], et[0:csz, :],
                                                     mask_t[0:csz, j, :])
                                exps.append(et)
                            if pending is not None:
                                att_tail(*pending)
                            pending = (h, b, exps)
                            if idx % 16 == 15:
                                emit_ada_group(2 + idx // 16)
                    att_tail(*pending)

            # v_pool freed
            # ---- out_proj + gate + residual (+ fused LN2 stats) ----
            with tc.tile_pool(name="xm2_pool", bufs=1) as xm2_pool:
                xm2T = xm2_pool.tile([P, DT, NINT], MM)

                with tc.tile_pool(name="x2_pool", bufs=1) as x2_pool:
                    x2T = x2_pool.tile([P, DT, NINT], MM)

                    with tc.tile_pool(name="o_sb_pool", bufs=1) as o_sb_pool, \
                         tc.tile_pool(name="wo_pool", bufs=1) as wop, \
                         tc.tile_pool(name="xi_pool", bufs=2) as xip, \
                         tc.tile_pool(name="f_scr", bufs=3) as fscr, \
                         tc.tile_pool(name="ln2_rows", bufs=1) as rows2, \
                         tc.tile_pool(name="ps_f", bufs=4, space="PSUM") as ps_f, \
                         tc.tile_pool(name="ps_st2", bufs=1, space="PSUM") as pst2:
                        wo_all = wop.tile([P, DT, D], MM)
                        for k in range(DT):
                            nc.sync.dma_start(
                                out=wo_all[:, k, :], in_=woT_d[k * P:(k + 1) * P, :])
                        o_sb = o_sb_pool.tile([P, DT, NINT], MM)
                        for k in range(DT):
                            nc.sync.dma_start(
                                out=o_sb[:, k, :], in_=oD[k * P:(k + 1) * P, :])
                        srow2 = rows2.tile([1, NINT], MM, tag="srow")
                        qrow2 = rows2.tile([1, NINT], MM, tag="qrow")
                        for b in range(B):
                            sl = slice(b * SS, (b + 1) * SS)
                            ssp = pst2.tile([1, SS], F32, tag="s2")
                            qsp = pst2.tile([1, SS], F32, tag="q2")
                            xi = xip.tile([P, DT, SS], MM, tag="xi")
                            nc.sync.dma_start(
                                out=xi[:],
                                in_=xT_d[:, b * SE + W:b * SE + W + SS]
                                .rearrange("(m p) s -> p m s", p=P))
                            def f_stats(m):
                                sq2 = fscr.tile([P, SS], MM, tag="sq2")
                                nc.vector.tensor_mul(sq2[:], x2T[:, m, sl],
                                                     x2T[:, m, sl])
                                nc.tensor.matmul(
                                    out=ssp[:], lhsT=R(ones_t[:, 0:1]),
                                    rhs=R(x2T[:, m, sl]),
                                    start=(m == 0), stop=(m == DT - 1))
                                nc.tensor.matmul(
                                    out=qsp[:], lhsT=R(ones_t[:, 0:1]),
                                    rhs=R(sq2[:]),
                                    start=(m == 0), stop=(m == DT - 1))
                            for m in range(DT):
                                pp = ps_f.tile([P, SS], F32, tag="op")
                                for k in range(DT):
                                    nc.tensor.matmul(
                                        out=pp[:],
                                        lhsT=R(wo_all[:, k, m * P:(m + 1) * P]),
                                        rhs=R(o_sb[:, k, sl]),
                                        start=(k == 0), stop=(k == DT - 1))
                                t1 = fscr.tile([P, SS], F32, tag="t1")
                                nc.scalar.activation(t1[:], pp[:], AF.Identity,
                                                     bias=ob_t[:, m:m + 1])
                                nc.vector.tensor_scalar_mul(t1[:], t1[:],
                                                            modcol(2, m, b))
                                nc.vector.tensor_add(x2T[:, m, sl], t1[:],
                                                     xi[:, m, :])
                                nc.sync.dma_start(
                                    out=x2D[m * P:(m + 1) * P, sl], in_=x2T[:, m, sl])
                                if m > 0:
                                    f_stats(m - 1)
                            f_stats(DT - 1)
                            nc.scalar.copy(srow2[:, sl], ssp[:])
                            nc.scalar.copy(qrow2[:, sl], qsp[:])
                        # ---- LN2 finalize + modulate -> xm2T ----
                        t12, mb2 = ln_finalize(srow2, qrow2, NINT, rows2)
                        for dti in range(DT):
                            eng = nc.vector
                            eng.tensor_mul(xm2T[:, dti, :], x2T[:, dti, :],
                                           t12[:])
                            eng.tensor_sub(xm2T[:, dti, :], xm2T[:, dti, :],
                                           mb2[:])
                            for b in range(B):
                                sl = slice(b * SS, (b + 1) * SS)
                                nc.scalar.activation(
                                    xm2T[:, dti, sl], xm2T[:, dti, sl], AF.Identity,
                                    bias=modcol(3, dti, b),
                                    scale=modcol(4, dti, b))

                # x2_pool freed; ---- MLP over hidden halves, SBUF accum ----
                with tc.tile_pool(name="h_pool", bufs=1) as hp, \
                     tc.tile_pool(name="yacc_pool", bufs=1) as yaccp, \
                     tc.tile_pool(name="w1_pool", bufs=2) as w1p, \
                     tc.tile_pool(name="w2_pool", bufs=2) as w2p, \
                     tc.tile_pool(name="mlp_scr", bufs=3) as mscr, \
                     tc.tile_pool(name="x2r_pool", bufs=2) as x2rp, \
                     tc.tile_pool(name="ps_h", bufs=4, space="PSUM") as ps_h:
                    y_acc = yaccp.tile([P, DT, NINT], F32)
                    for hg in range(2):
                        hT = hp.tile([P, 16, NINT], MM, tag="h")
                        for hm in range(16):
                            hmg = hg * 16 + hm
                            wm = w1p.tile([P, DT, P], MM, tag="w1")
                            nc.sync.dma_start(
                                out=wm[:],
                                in_=w1T_d[:, hmg * P:(hmg + 1) * P]
                                .rearrange("(k p) m -> p k m", p=P))
                            for nci in range(2):
                                ph = ps_h.tile([P, 512], F32, tag="h1")
                                for k in range(DT):
                                    nc.tensor.matmul(
                                        out=ph[:],
                                        lhsT=R(wm[:, k, :]),
                                        rhs=R(xm2T[:, k, nci * 512:(nci + 1) * 512]),
                                        start=(k == 0), stop=(k == DT - 1))
                                nc.scalar.activation(
                                    hT[:, hm, nci * 512:(nci + 1) * 512], ph[:],
                                    AF.Relu, bias=b1_t[:, hmg:hmg + 1])
                        for m in range(DT):
                            wm2 = w2p.tile([P, 16, P], MM, tag="w2")
                            nc.sync.dma_start(
                                out=wm2[:],
                                in_=w2T_d[hg * 2048:(hg + 1) * 2048, m * P:(m + 1) * P]
                                .rearrange("(k p) m -> p k m", p=P))
                            for nci in range(2):
                                pm2 = ps_h.tile([P, 512], F32, tag="h2")
                                for kk in range(16):
                                    nc.tensor.matmul(
                                        out=pm2[:],
                                        lhsT=R(wm2[:, kk, :]),
                                        rhs=R(hT[:, kk, nci * 512:(nci + 1) * 512]),
                                        start=(kk == 0), stop=(kk == 15))
                                sl = slice(nci * 512, (nci + 1) * 512)
                                if hg == 0:
                                    nc.scalar.activation(
                                        y_acc[:, m, sl], pm2[:], AF.Identity,
                                        bias=b2_t[:, m:m + 1])
                                else:
                                    nc.vector.tensor_add(
                                        y_acc[:, m, sl], y_acc[:, m, sl], pm2[:])
                    # ---- gate + residual + store ----
                    for m in range(DT):
                        x2r = x2rp.tile([P, NINT], MM, tag="x2r")
                        nc.sync.dma_start(out=x2r[:],
                                          in_=x2D[m * P:(m + 1) * P, :])
                        yt = mscr.tile([P, NINT], F32, tag="yt")
                        for b in range(B):
                            sl = slice(b * SS, (b + 1) * SS)
                            nc.vector.tensor_scalar_mul(
                                yt[:, sl], y_acc[:, m, sl], modcol(5, m, b))
                        nc.vector.tensor_add(yt[:], yt[:], x2r[:])
                        nc.sync.dma_start(out=yT_d[m * P:(m + 1) * P, :], in_=yt[:])

    nc.compile()
    return nc


def _host_inputs(x, c, ada_w, ada_b, in_proj_w, in_proj_b, out_proj_w,
                 out_proj_b, mlp_w1, mlp_b1, mlp_w2, mlp_b2):
    f = np.float32
    x = np.asarray(x, f)

    # shared (identical on every core)
    ada_b_adj = np.asarray(ada_b, f).copy()
    ada_b_adj[D:2 * D] += 1.0      # fold the (1 + scale) into the bias
    ada_b_adj[4 * D:5 * D] += 1.0
    adaTa = np.concatenate([np.asarray(ada_w, f).T,
                            ada_b_adj[None, :]], axis=0)
    cTa = np.concatenate([np.asarray(c, f).T,
                          np.ones((1, B), f)], axis=0)
    shared = {
        'cTa': np.ascontiguousarray(cTa),
        'adaTa': np.ascontiguousarray(adaTa),
        'wqkT': np.ascontiguousarray(np.asarray(in_proj_w, f)[:2 * D].T),
        'qkb': np.ascontiguousarray(np.asarray(in_proj_b, f)[:2 * D]),
        'wvT': np.ascontiguousarray(np.asarray(in_proj_w, f)[2 * D:].T),
        'vb128': np.ascontiguousarray(
            np.broadcast_to(np.asarray(in_proj_b, f)[2 * D:], (P, D))),
        'woT': np.ascontiguousarray(np.asarray(out_proj_w, f).T),
        'ob': np.ascontiguousarray(np.asarray(out_proj_b, f)),
        'w1T': np.ascontiguousarray(np.asarray(mlp_w1, f).T),
        'b1': np.ascontiguousarray(np.asarray(mlp_b1, f)),
        'w2T': np.ascontiguousarray(np.asarray(mlp_w2, f).T),
        'b2': np.ascontiguousarray(np.asarray(mlp_b2, f)),
    }

    in_maps = []
    band_k = np.arange(SE)[:, None]           # local key index
    band_q = np.arange(SS)[None, :]           # local query index
    band = ((band_k - band_q >= 0) & (band_k - band_q <= 2 * W))
    for i in range(NCORES):
        s_lo = i * SS - W
        x_ext = np.zeros((SE, B, D), f)
        lo = max(0, s_lo)
        hi = min(S, s_lo + SE)
        x_ext[lo - s_lo:hi - s_lo] = x[lo:hi]
        xT = np.ascontiguousarray(x_ext.transpose(2, 1, 0).reshape(D, NEXT))

        s_glob = s_lo + np.arange(SE)
        valid = (s_glob >= 0) & (s_glob < S)
        mask = (band & valid[:, None]).astype(f)

        mean = x_ext.astype(np.float64).mean(-1)           # [SE, B]
        var = x_ext.astype(np.float64).var(-1)
        rstd = 1.0 / np.sqrt(var + EPS)
        lnrows = np.stack([rstd, mean * rstd]).astype(f)   # [2, SE, B]
        lnrows = lnrows.transpose(0, 2, 1).reshape(2, NEXT)  # b-major tokens

        m = dict(shared)
        m['xT'] = xT
        m['mask'] = np.ascontiguousarray(mask)
        m['lnrows'] = np.ascontiguousarray(lnrows)
        in_maps.append(m)
    return in_maps


def kernel(x, c, ada_w, ada_b, in_proj_w, in_proj_b, out_proj_w, out_proj_b,
           mlp_w1, mlp_b1, mlp_w2, mlp_b2, num_heads, window_size):
    assert int(num_heads) == H and int(window_size) == W

    from concourse.bass_utils import run_bass_kernel_spmd

    if 'nc' not in _BUILD_CACHE:
        _BUILD_CACHE['nc'] = _build_nc()
    nc = _BUILD_CACHE['nc']

    in_maps = _host_inputs(x, c, ada_w, ada_b, in_proj_w, in_proj_b,
                           out_proj_w, out_proj_b, mlp_w1, mlp_b1,
                           mlp_w2, mlp_b2)
    res = run_bass_kernel_spmd(nc, in_maps, list(range(NCORES)))

    out = np.empty((S, B, D), np.float32)
    for i in range(NCORES):
        yT = res.results[i]['yT']                      # [D, NINT]
        y = yT.reshape(D, B, SS).transpose(2, 1, 0)    # [SS, B, D]
        out[i * SS:(i + 1) * SS] = y
    return out

